# revision 31
# baseline (speedup 1.0000x reference)
"""CoAttention kernel for Trainium2, 8 NeuronCores, data-parallel over batch.

Reference computation (per batch b):
    k_proj = key @ W_k.T + b_k            # (S, D)
    scores = query @ k_proj.T             # (S, S)
    scores += log(cell_mask) + log(seq_mask)[None, :]
    p = softmax(scores, axis=-1)
    out = (p @ value) @ W_o.T + b_o       # (S, D)

Algebraic simplifications used (exact):
  - scores = query @ W_k @ key.T + (query @ b_k)[:, None]; the b_k term is
    constant along the softmax axis, so softmax is invariant to it -> b_k
    is dropped entirely.
  - cell_mask/seq_mask are all-ones per the problem spec (log == 0); the
    kernel checks this on the host and falls back to a numpy path if not.
  - b_o is added on the host (it is all-zeros per spec, but handled exactly).

Precision scheme (default, SCORES_MODE="f32r"):
  - scores path (q_proj = query @ W_k, scores = q_proj @ key.T): float32r
    single-pass matmuls (TF32-like reduced-mantissa fp32) at full PE rate
    (1 cycle/row, vs 4 for fp32 and 3 passes for the f16x3 fallback).
    Rounding to f32r happens on the PSUM->SBUF evacuation copies (DVE);
    q/k tiles are PE-transposed in plain fp32 during the DMA-bound
    startup phases where PE has slack.
  - softmax: fp32 row max (negated) on DVE, exp on ScalarE with fused
    row-sum; 1/rowsum applied on the x evacuation.
  - tail (p @ value, x @ W_o.T): fp16 operands, fp32 PSUM accumulation.
    value / W_o are pre-cast to fp16 on the host (halves their DMA).
Measured absmax error vs fp64 reference ~7.7e-3 relative to output scale
(gate is 2e-2); the f16x3 mode (~3e-4) remains available via
KERNEL_SCORES_MODE=f16x3 at ~1.8x the runtime.

Schedule: phase 0a overlaps W_k (gpsimd DMA queue) and query loads (sync
queue) with fp32 q transposes and the q_projT matmul chunks; phase 0b
overlaps key loads/transposes with the scores matmuls of q-block 0 while
value/W_o stream in on the gpsimd queue. The main loop emits, per block:
scores(qb) / pT-transposes(qb-1) / max+exp(qb) / x+out tail(qb-1) /
recip(qb), so ~11us of tail PE work covers the max+exp latency before
scores(qb+1) needs the single scores-PSUM buffer, and the in-order DVE
queue (max, xn evacs, out evacs, recip) never blocks PE. The pT-transpose
staging and x-accumulation PSUM tiles share one 2-buffer ring (their
lifetimes interleave), double-buffering both within the 8-bank budget.
"""

import os as _os

import numpy as np

import concourse.bass as bass
import concourse.mybir as mybir
import concourse.tile as tile
from concourse import bacc
from concourse.bass_utils import run_bass_kernel_spmd
from concourse.masks import make_identity

P = 128
S = 2048
D = 1024
NBS = S // P   # 16 row blocks of seq
NBD = D // P   # 8 row blocks of feature dim
NC = 8         # cores == batch
F32 = mybir.dt.float32
F16 = mybir.dt.float16
F32R = mybir.dt.float32r

# scores-path mode: "f16x3" (safe, 3 passes) or "f32r" (fast, 1 pass)
SCORES_MODE = _os.environ.get("KERNEL_SCORES_MODE", "f32r")
# tail mode: "xt" (v-stationary, no x transpose) or "tr" (x + transpose)
TAIL_MODE = _os.environ.get("KERNEL_TAIL_MODE", "tr")


def build_nc(scores_mode=SCORES_MODE, tail_mode=TAIL_MODE, repeat=1):
    nc = bacc.Bacc("TRN2", target_bir_lowering=False, debug=False)
    d_query = nc.dram_tensor("query", [S, D], F32, kind="ExternalInput")
    d_key = nc.dram_tensor("key", [S, D], F32, kind="ExternalInput")
    # value / W_o only feed fp16 matmuls -> host pre-casts them to fp16,
    # halving their DMA and letting them land directly in resident tiles.
    d_value = nc.dram_tensor("value16", [S, D], F16, kind="ExternalInput")
    d_wk = nc.dram_tensor("W_k", [D, D], F32, kind="ExternalInput")
    d_wo = nc.dram_tensor("W_o16", [D, D], F16, kind="ExternalInput")
    d_out = nc.dram_tensor("out", [S, D], F32, kind="ExternalOutput")

    if scores_mode == "f16x3":
        s_dt = F16
        passes = [(0, 0), (0, 1), (1, 0)]  # (lhs comp, rhs comp) over [hi, lo]
        ncomp = 2
    elif scores_mode == "f32r":
        s_dt = F32R
        passes = [(0, 0)]
        ncomp = 1
    else:
        raise ValueError(scores_mode)

    def split(hi, lo, src):
        """hi = ACT cast; lo = DVE (src - hi) rounded."""
        nc.scalar.copy(hi, src)
        nc.vector.tensor_sub(lo, src, hi)

    with tile.TileContext(nc) as tc:
      def emit_body():
            # ---------------- constants ----------------
            const_pool = tc.alloc_tile_pool(name="const", bufs=1)
            ident16 = const_pool.tile([P, P], F16)
            make_identity(nc, ident16[:])
            ident32 = const_pool.tile([P, P], F32)
            make_identity(nc, ident32[:])
            if ncomp == 1:
                ident_r = const_pool.tile([P, P], s_dt)
                nc.vector.tensor_copy(ident_r[:], ident32[:])
                id_s = ident_r
            else:
                id_s = ident16

            def tr8(ps_pool, dst3d, src2d, qi, ident, dt, tag="tp", copy_eng="scalar"):
                """Transpose NBD 128x128 blocks of src2d [P, D] into column qi of
                dst3d [P, NBD, cols] via PSUM + strided copies. 4-byte dtypes
                split into two half-tiles so each stays within one PSUM bank."""
                gsz = NBD if dt in (F16, mybir.dt.bfloat16) else NBD // 2
                for g0 in range(0, NBD, gsz):
                    t = ps_pool.tile([P, gsz * P], dt, tag=tag,
                                     name=f"t_{tag}_{qi}_{g0}")
                    for j in range(gsz):
                        nc.tensor.transpose(t[:, j * P:(j + 1) * P],
                                            src2d[:, (g0 + j) * P:(g0 + j + 1) * P],
                                            ident[:])
                    dst = dst3d[:, g0:g0 + gsz, qi * P:(qi + 1) * P]
                    src = t[:].rearrange("p (j q) -> p j q", j=gsz)
                    if copy_eng == "scalar":
                        nc.scalar.copy(dst, src)
                    else:
                        nc.vector.tensor_copy(dst, src)

            # ---------------- resident: q_projT ----------------
            # qpT[c]: [P, NBD*S]; block db at columns [db*S, (db+1)*S)
            qpT_pool = tc.alloc_tile_pool(name="qpT", bufs=1)
            qpT = [qpT_pool.tile([P, NBD * S], s_dt, name=f"qpT{c}")
                   for c in range(ncomp)]

            # ============ phase 0a: query transpose + W_k + q_projT, interleaved ====
            with tc.tile_pool(name="p0a_sb", bufs=3 if ncomp == 2 else 6) as p0a_sb, \
                 tc.tile_pool(name="p0a_wk", bufs=1) as p0a_wk, \
                 tc.tile_pool(name="p0a_qt", bufs=1) as p0a_qt, \
                 tc.tile_pool(name="p0a_ps", bufs=2, space="PSUM") as p0a_ps, \
                 tc.tile_pool(name="p0a_ps2", bufs=4, space="PSUM") as p0a_ps2:

                qt_c = [p0a_qt.tile([P, NBD * S], s_dt, name=f"qt{c}")
                        for c in range(ncomp)]
                qt3 = [t[:].rearrange("p (j s) -> p j s", j=NBD) for t in qt_c]
                wk_c = [[p0a_wk.tile([P, D], s_dt, name=f"wk{c}_{i}")
                         for i in range(NBD)]
                        for c in range(ncomp)]

                def do_query_tile(qi):
                    q_f32 = p0a_sb.tile([P, D], F32, tag="ld32", name=f"qld{qi}")
                    nc.sync.dma_start(q_f32[:], d_query[qi * P:(qi + 1) * P, :])
                    if ncomp == 2:
                        qh = p0a_sb.tile([P, D], F16, tag="q_hi", name=f"qh{qi}")
                        ql = p0a_sb.tile([P, D], F16, tag="q_lo", name=f"ql{qi}")
                        split(qh[:], ql[:], q_f32[:])
                        for c, src in enumerate([qh[:], ql[:]]):
                            tr8(p0a_ps, qt3[c], src, qi, id_s, s_dt,
                                copy_eng="vector" if c else "scalar")
                    else:
                        # transpose in fp32 (PE has slack here); the f32r
                        # rounding happens on the PSUM->SBUF convert copy
                        tr8(p0a_ps, qt3[0], q_f32[:], qi, ident32, F32,
                            copy_eng="vector")

                def do_qp_chunk(qc):
                    # q_projT[d, qc-cols] = sum_{d'} W_k[d', d] * QT[d', qc-cols]
                    for db in range(NBD):
                        ps = p0a_ps2.tile([P, 512], F32, tag="qp",
                                          name=f"qp{db}_{qc}")
                        n_acc = len(passes) * NBD
                        idx = 0
                        for (lc, rc) in passes:
                            for dpb in range(NBD):
                                nc.tensor.matmul(
                                    ps[:],
                                    wk_c[lc][dpb][:, db * P:(db + 1) * P],
                                    qt_c[rc][:, dpb * S + qc * 512:
                                              dpb * S + (qc + 1) * 512],
                                    start=(idx == 0), stop=(idx == n_acc - 1))
                                idx += 1
                        off = db * S + qc * 512
                        if ncomp == 2:
                            split(qpT[0][:, off:off + 512],
                                  qpT[1][:, off:off + 512], ps[:])
                        else:
                            nc.vector.tensor_copy(qpT[0][:, off:off + 512], ps[:])

                # W_k first (chunk 0 needs all of it), on the gpsimd queue so
                # it streams in parallel with the query loads on sync's queue.
                for i in range(NBD):
                    wk_f32 = p0a_sb.tile([P, D], F32, tag="ldwk",
                                         name=f"wkld{i}", bufs=3)
                    nc.gpsimd.dma_start(wk_f32[:], d_wk[i * P:(i + 1) * P, :])
                    if ncomp == 2:
                        split(wk_c[0][i][:], wk_c[1][i][:], wk_f32[:])
                    else:
                        nc.vector.tensor_copy(wk_c[0][i][:], wk_f32[:])
                for qi in range(4):
                    do_query_tile(qi)
                for qc in range(4):
                    do_qp_chunk(qc)
                    if qc < 3:
                        for qi in range(4 * (qc + 1), 4 * (qc + 2)):
                            do_query_tile(qi)

            # scores PSUM lives from phase 0b (first q-block overlap) onward
            sc_ps = tc.alloc_tile_pool(name="sc_ps", bufs=1, space="PSUM")

            # ---------------- resident: keyT, value, W_oT ----------------
            kT_pool = tc.alloc_tile_pool(name="kT", bufs=1)
            v_pool = tc.alloc_tile_pool(name="v", bufs=1)
            wo_pool = tc.alloc_tile_pool(name="wo", bufs=1)
            kT = [kT_pool.tile([P, NBD * S], s_dt, name=f"kT{c}")
                  for c in range(ncomp)]
            kT3 = [t[:].rearrange("p (j s) -> p j s", j=NBD) for t in kT]
            vv = [v_pool.tile([P, D], F16, name=f"v_{i}") for i in range(NBS)]
            woT = wo_pool.tile([P, NBD * D], F16, name="woT")
            woT3 = woT[:].rearrange("p (j o) -> p j o", j=NBD)

            # softmax-state pools (used from phase 0b for q-block 0)
            exp_sb = tc.alloc_tile_pool(name="exp_sb", bufs=2)
            st_sb = tc.alloc_tile_pool(name="st_sb", bufs=2)

            state = {}

            def head_mm_chunk(qb, kc, scores, batch=None):
                """Emit the scores matmuls for 512-col chunk kc of q-block qb.

                batch=None emits all len(passes)*NBD accumulating matmuls;
                batch=i emits only pass i's NBD matmuls (same accumulation
                group, split for interleaving as PE filler inside tail())."""
                q0 = qb * P
                n_acc = len(passes) * NBD
                for pi, (lc, rc) in enumerate(passes):
                    if batch is not None and pi != batch:
                        continue
                    for db in range(NBD):
                        idx = pi * NBD + db
                        nc.tensor.matmul(
                            scores[:, kc * 512:(kc + 1) * 512],
                            qpT[lc][:, db * S + q0:db * S + q0 + P],
                            kT[rc][:, db * S + kc * 512:db * S + (kc + 1) * 512],
                            start=(idx == 0), stop=(idx == n_acc - 1))

            def head_mm(qb, chunks=None):
                if qb not in state:
                    state[qb] = {"scores": sc_ps.tile([P, S], F32, tag="scores",
                                                      name=f"scores{qb}")}
                scores = state[qb]["scores"]
                for kc in (range(S // 512) if chunks is None else chunks):
                    head_mm_chunk(qb, kc, scores)

            def head_softmax(qb, max_eng="vector"):
                """Row max (negated) + exp-with-rowsum. The reciprocal is
                emitted separately (head_recip) so it can sit AFTER the
                previous block's DVE evacuations in the in-order DVE queue."""
                st = state[qb]
                scores = st["scores"]
                neg_max = st_sb.tile([P, 1], F32, tag="negmax", name=f"negmax{qb}")
                eng = nc.gpsimd if max_eng == "pool" else nc.vector
                eng.reduce_max(neg_max[:], scores[:],
                               axis=mybir.AxisListType.X, negate=True)
                rowsum = st_sb.tile([P, 1], F32, tag="rowsum", name=f"rowsum{qb}")
                expv = exp_sb.tile([P, S], F16, tag="expv", name=f"expv{qb}")
                nc.scalar.activation(expv[:], scores[:],
                                     mybir.ActivationFunctionType.Exp,
                                     bias=neg_max[:], scale=1.0,
                                     accum_out=rowsum[:])
                st["expv"] = expv
                st["rowsum"] = rowsum

            def head_recip(qb):
                st = state[qb]
                recip = st_sb.tile([P, 1], F32, tag="recip", name=f"recip{qb}")
                nc.vector.reciprocal(recip[:], st["rowsum"][:])
                st["recip"] = recip

            # ============ phase 0b: keyT build overlapped with scores(0) ============
            with tc.tile_pool(name="p0b_sb", bufs=3 if ncomp == 1 else 2) as p0b_sb, \
                 tc.tile_pool(name="p0b_ps", bufs=2, space="PSUM") as p0b_ps:

                scores0 = sc_ps.tile([P, S], F32, tag="scores", name="scores_0")
                state[0] = {"scores": scores0}

                def do_wo_tile(oi):
                    wo_f16 = p0b_sb.tile([P, D], F16, tag="ld16", name=f"wold{oi}")
                    nc.gpsimd.dma_start(wo_f16[:], d_wo[oi * P:(oi + 1) * P, :])
                    t16 = p0b_ps.tile([P, NBD * P], F16, tag="tp16",
                                      name=f"twoT{oi}", bufs=1)
                    for j in range(NBD):
                        nc.tensor.transpose(t16[:, j * P:(j + 1) * P],
                                            wo_f16[:, j * P:(j + 1) * P],
                                            ident16[:])
                    nc.scalar.copy(woT3[:, :, oi * P:(oi + 1) * P],
                                   t16[:].rearrange("p (j q) -> p j q", j=NBD))

                # value lands directly in its resident fp16 tiles via the
                # gpsimd queue, in parallel with key loads on sync's queue.
                for ki in range(NBS):
                    nc.gpsimd.dma_start(vv[ki][:], d_value[ki * P:(ki + 1) * P, :])

                for kc in range(4):
                    for ki in range(4 * kc, 4 * (kc + 1)):
                        k_f32 = p0b_sb.tile([P, D], F32, tag="ld32",
                                            name=f"kld{ki}")
                        nc.sync.dma_start(k_f32[:], d_key[ki * P:(ki + 1) * P, :])
                        if ncomp == 2:
                            kh = p0b_sb.tile([P, D], F16, tag="k_hi",
                                             name=f"kh{ki}")
                            kl = p0b_sb.tile([P, D], F16, tag="k_lo",
                                             name=f"kl{ki}")
                            split(kh[:], kl[:], k_f32[:])
                            for c, src in enumerate([kh[:], kl[:]]):
                                tr8(p0b_ps, kT3[c], src, ki, id_s, s_dt,
                                    copy_eng="vector" if c else "scalar")
                        else:
                            tr8(p0b_ps, kT3[0], k_f32[:], ki, ident32, F32,
                                copy_eng="vector")
                    head_mm_chunk(0, kc, scores0)

                head_softmax(0)
                head_recip(0)

                for oi in range(NBD):
                    do_wo_tile(oi)

            # ============ main loop over q blocks (software-pipelined) ============
            # ptp (2KB f16) and xp (2KB f32) share one 2-buffer PSUM ring:
            # their lifetimes interleave (ptp g0/g1 -> xp dh0/dh1), so two
            # banks double-buffer both, and PSUM stays at 8 banks total.
            trx_ps = tc.alloc_tile_pool(name="trx_ps", bufs=2, space="PSUM")
            tr_ps = trx_ps
            x_ps = trx_ps
            o_ps = tc.alloc_tile_pool(name="o_ps", bufs=2, space="PSUM")
            pt_sb = tc.alloc_tile_pool(name="pt_sb", bufs=4)
            xt_sb = tc.alloc_tile_pool(name="xt_sb", bufs=2)
            out_sb = tc.alloc_tile_pool(name="out_sb", bufs=2)

            def tail_pt(qb):
                """Transpose exp(scores) for q-block qb into pT (k on
                partitions): 8 blocks per PSUM bank, evacuated on ACT."""
                st = state[qb]
                expv = st["expv"]
                pts = []
                for g in range(2):
                    ptp = tr_ps.tile([P, 8 * P], F16, tag="trx",
                                     name=f"ptp{qb}_{g}")
                    for j in range(8):
                        kb = g * 8 + j
                        nc.tensor.transpose(
                            ptp[:, j * P:(j + 1) * P],
                            expv[:, kb * P:(kb + 1) * P], ident16[:])
                    pt = pt_sb.tile([P, 8 * P], F16, tag="pt", name=f"pt{qb}_{g}")
                    nc.scalar.copy(pt[:], ptp[:])
                    pts.append(pt)
                st["pts"] = pts

            def tail_rest_xt(qb, filler=None):
                """xT = V.T-contracted with pT (no transpose needed: v is the
                stationary operand in natural [k, d] layout), then
                out[q, o] = xT.T @ woT with the 1/rowsum folded into the
                final PSUM evacuation (q is the partition dim there)."""
                st = state.pop(qb)
                pts, recip = st["pts"], st["recip"]

                # xT[d, q] in two halves of d; 4 d-blocks per PSUM bank
                xts = []
                for h in range(2):
                    xp = x_ps.tile([P, 4 * P], F32, tag="trx",
                                   name=f"xp{qb}_{h}")
                    for dj in range(4):
                        db = h * 4 + dj
                        for kb in range(NBS):
                            nc.tensor.matmul(
                                xp[:, dj * P:(dj + 1) * P],
                                vv[kb][:, db * P:(db + 1) * P],
                                pts[kb // 8][:, (kb % 8) * P:(kb % 8 + 1) * P],
                                start=(kb == 0), stop=(kb == NBS - 1))
                    xt = xt_sb.tile([P, 4 * P], F16, tag="xt",
                                    name=f"xt{qb}_{h}")
                    nc.scalar.copy(xt[:], xp[:])
                    xts.append(xt)
                    if h == 0 and filler is not None:
                        filler()

                # out = xT.T @ woT, o in halves; scale by recip on evacuation
                ops = [o_ps.tile([P, 512], F32, tag="op", name=f"op{qb}_{i}")
                       for i in range(2)]
                for db in range(NBD):
                    lhs = xts[db // 4][:, (db % 4) * P:(db % 4 + 1) * P]
                    for oh in range(2):
                        nc.tensor.matmul(
                            ops[oh][:], lhs,
                            woT[:, db * D + oh * 512:db * D + (oh + 1) * 512],
                            start=(db == 0), stop=(db == NBD - 1))
                q0 = qb * P
                for oh in range(2):
                    osb = out_sb.tile([P, 512], F32, tag="osb",
                                      name=f"osb{qb}_{oh}")
                    nc.vector.tensor_scalar_mul(osb[:], ops[oh][:], recip[:])
                    nc.sync.dma_start(
                        d_out[q0:q0 + P, oh * 512:(oh + 1) * 512], osb[:])

            def tail_rest_tr(qb, filler=None):
                """Classic tail: x = pT.T @ v (512-col moving dim), normalize
                on evacuation, PE-transpose x, then out = xT.T @ woT.
                `filler` emits PE work (next block's scores chunks) after the
                first x half to cover the single-buffer PSUM evacuation."""
                st = state.pop(qb)
                pts, recip = st["pts"], st["recip"]
                xn = xt_sb.tile([P, D], F16, tag="xn", name=f"xn{qb}")
                for dh in range(2):
                    xp = x_ps.tile([P, 512], F32, tag="trx", name=f"xp{qb}_{dh}")
                    for kb in range(NBS):
                        nc.tensor.matmul(
                            xp[:],
                            pts[kb // 8][:, (kb % 8) * P:(kb % 8 + 1) * P],
                            vv[kb][:, dh * 512:(dh + 1) * 512],
                            start=(kb == 0), stop=(kb == NBS - 1))
                    nc.vector.tensor_scalar_mul(
                        xn[:, dh * 512:(dh + 1) * 512], xp[:], recip[:])
                    if dh == 0 and filler is not None:
                        filler()
                xtp = tr_ps.tile([P, 8 * P], F16, tag="trx", name=f"xtp{qb}")
                for j in range(NBD):
                    nc.tensor.transpose(xtp[:, j * P:(j + 1) * P],
                                        xn[:, j * P:(j + 1) * P], ident16[:])
                xt = xt_sb.tile([P, 8 * P], F16, tag="xt", name=f"xt{qb}")
                nc.scalar.copy(xt[:], xtp[:])
                ops = [o_ps.tile([P, 512], F32, tag="op", name=f"op{qb}_{i}")
                       for i in range(2)]
                for db in range(NBD):
                    lhs = xt[:, db * P:(db + 1) * P]
                    for oh in range(2):
                        nc.tensor.matmul(
                            ops[oh][:], lhs,
                            woT[:, db * D + oh * 512:db * D + (oh + 1) * 512],
                            start=(db == 0), stop=(db == NBD - 1))
                q0 = qb * P
                for oh in range(2):
                    osb = out_sb.tile([P, 512], F32, tag="osb",
                                      name=f"osb{qb}_{oh}")
                    nc.vector.tensor_copy(osb[:], ops[oh][:])
                    nc.sync.dma_start(
                        d_out[q0:q0 + P, oh * 512:(oh + 1) * 512], osb[:])

            tail_rest = tail_rest_xt if tail_mode == "xt" else tail_rest_tr

            for qb in range(1, NBS + 1):
                if qb < NBS:
                    head_mm(qb)
                    tail_pt(qb - 1)
                    # DVE max first (its scores input is ready before the
                    # tail's evacuations need DVE), reciprocal emitted after
                    # the tail so it can't head-of-line-block the DVE queue.
                    head_softmax(qb)
                    tail_rest(qb - 1)
                    head_recip(qb)
                else:
                    tail_pt(qb - 1)
                    tail_rest(qb - 1)

            out_sb.release()
            xt_sb.release()
            pt_sb.release()
            o_ps.release()
            trx_ps.release()
            st_sb.release()
            exp_sb.release()
            wo_pool.release()
            v_pool.release()
            kT_pool.release()
            sc_ps.release()
            qpT_pool.release()
            const_pool.release()


      for _rep in range(repeat):
          emit_body()

    nc.compile()
    return nc


_NC_CACHE = {}


def _get_nc():
    if "nc" not in _NC_CACHE:
        _NC_CACHE["nc"] = build_nc()
    return _NC_CACHE["nc"]


def make_in_maps(query, key, value, W_k, W_o):
    value16 = value.astype(np.float16)
    W_o16 = W_o.astype(np.float16)
    return [
        {"query": query[b], "key": key[b], "value16": value16[b],
         "W_k": W_k, "W_o16": W_o16}
        for b in range(NC)
    ]


def _numpy_fallback(query, key, value, cell_mask, seq_mask, W_k, b_k, W_o, b_o):
    out = np.empty((query.shape[0], S, D), dtype=np.float32)
    for b in range(query.shape[0]):
        kp = key[b].astype(np.float64) @ W_k.astype(np.float64).T + b_k
        s = query[b].astype(np.float64) @ kp.T
        s = s + np.log(cell_mask[b]) + np.log(seq_mask[b])[None, :]
        s -= s.max(1, keepdims=True)
        e = np.exp(s)
        p = e / e.sum(1, keepdims=True)
        x = p @ value[b].astype(np.float64)
        out[b] = (x @ W_o.astype(np.float64).T + b_o).astype(np.float32)
    return out


def kernel(query, key, value, cell_mask, seq_mask, W_k, b_k, W_o, b_o):
    query = np.ascontiguousarray(query, dtype=np.float32)
    key = np.ascontiguousarray(key, dtype=np.float32)
    value = np.ascontiguousarray(value, dtype=np.float32)
    W_k = np.ascontiguousarray(W_k, dtype=np.float32)
    W_o = np.ascontiguousarray(W_o, dtype=np.float32)

    # masks are all-ones per the problem spec -> log-mask bias is exactly 0.
    # b_k shifts every score row by a constant -> softmax-invariant (exact).
    if not (np.all(np.asarray(cell_mask) == 1.0)
            and np.all(np.asarray(seq_mask) == 1.0)):
        return _numpy_fallback(np.asarray(query), np.asarray(key),
                               np.asarray(value), np.asarray(cell_mask),
                               np.asarray(seq_mask), W_k,
                               np.asarray(b_k), W_o, np.asarray(b_o))

    nc = _get_nc()
    in_maps = make_in_maps(query, key, value, W_k, W_o)
    res = run_bass_kernel_spmd(nc, in_maps, core_ids=list(range(NC)))
    out = np.stack([res.results[b]["out"] for b in range(NC)])
    if b_o is not None and np.any(np.asarray(b_o) != 0.0):
        out = out + np.asarray(b_o, dtype=np.float32)[None, None, :]
    return out



# revision 33
# speedup vs baseline: 1.2304x; 1.2304x over previous
"""CoAttention kernel for Trainium2, 8 NeuronCores, data-parallel over batch.

Reference computation (per batch b):
    k_proj = key @ W_k.T + b_k            # (S, D)
    scores = query @ k_proj.T             # (S, S)
    scores += log(cell_mask) + log(seq_mask)[None, :]
    p = softmax(scores, axis=-1)
    out = (p @ value) @ W_o.T + b_o       # (S, D)

Algebraic simplifications used (exact):
  - scores = query @ W_k @ key.T + (query @ b_k)[:, None]; the b_k term is
    constant along the softmax axis, so softmax is invariant to it -> b_k
    is dropped entirely.
  - cell_mask/seq_mask are all-ones per the problem spec (log == 0); the
    kernel checks this on the host and falls back to a numpy path if not.
  - b_o is added on the host (it is all-zeros per spec, but handled exactly).

Precision scheme (default, SCORES_MODE="f32r"):
  - scores path (q_proj = query @ W_k, scores = q_proj @ key.T): float32r
    single-pass matmuls (TF32-like reduced-mantissa fp32) at full PE rate
    (1 cycle/row, vs 4 for fp32 and 3 passes for the f16x3 fallback).
    Rounding to f32r happens on the PSUM->SBUF evacuation copies (DVE);
    q/k tiles are PE-transposed in plain fp32 during the DMA-bound
    startup phases where PE has slack.
  - softmax: fp32 row max (negated) on DVE, exp on ScalarE with fused
    row-sum; 1/rowsum applied on the x evacuation.
  - tail (p @ value, x @ W_o.T): fp16 operands, fp32 PSUM accumulation.
    value / W_o are pre-cast to fp16 on the host (halves their DMA).
Measured absmax error vs fp64 reference ~7.7e-3 relative to output scale
(gate is 2e-2); the f16x3 mode (~3e-4) remains available via
KERNEL_SCORES_MODE=f16x3 at ~1.8x the runtime.

Schedule: phase 0a overlaps W_k (gpsimd DMA queue) and query loads (sync
queue) with fp32 q transposes and the q_projT matmul chunks; phase 0b
overlaps key loads/transposes with the scores matmuls of q-block 0 while
value/W_o stream in on the gpsimd queue. The main loop emits, per block:
scores(qb) / pT-transposes(qb-1) / max+exp(qb) / x+out tail(qb-1) /
recip(qb), so ~11us of tail PE work covers the max+exp latency before
scores(qb+1) needs the single scores-PSUM buffer, and the in-order DVE
queue (max, xn evacs, out evacs, recip) never blocks PE. The pT-transpose
staging and x-accumulation PSUM tiles share one 2-buffer ring (their
lifetimes interleave), double-buffering both within the 8-bank budget.
"""

import os as _os

import numpy as np

import concourse.bass as bass
import concourse.mybir as mybir
import concourse.tile as tile
from concourse import bacc
from concourse.bass_utils import run_bass_kernel_spmd
from concourse.masks import make_identity

P = 128
S = 2048
D = 1024
NBS = S // P   # 16 row blocks of seq
NBD = D // P   # 8 row blocks of feature dim
NC = 8         # cores == batch
F32 = mybir.dt.float32
F16 = mybir.dt.float16
F32R = mybir.dt.float32r

# scores-path mode: "f16x3" (safe, 3 passes) or "f32r" (fast, 1 pass)
SCORES_MODE = _os.environ.get("KERNEL_SCORES_MODE", "f32r")
# tail mode: "xt" (v-stationary, no x transpose) or "tr" (x + transpose)
TAIL_MODE = _os.environ.get("KERNEL_TAIL_MODE", "tr")


def build_nc(scores_mode=SCORES_MODE, tail_mode=TAIL_MODE, repeat=1):
    nc = bacc.Bacc("TRN2", target_bir_lowering=False, debug=False)
    d_query = nc.dram_tensor("query", [S, D], F32, kind="ExternalInput")
    d_key = nc.dram_tensor("key", [S, D], F32, kind="ExternalInput")
    # value / W_o only feed fp16 matmuls -> host pre-casts them to fp16,
    # halving their DMA and letting them land directly in resident tiles.
    d_value = nc.dram_tensor("value16", [S, D], F16, kind="ExternalInput")
    d_wk = nc.dram_tensor("W_k", [D, D], F32, kind="ExternalInput")
    d_wo = nc.dram_tensor("W_o16", [D, D], F16, kind="ExternalInput")
    d_out = nc.dram_tensor("out", [S, D], F32, kind="ExternalOutput")

    if scores_mode == "f16x3":
        s_dt = F16
        passes = [(0, 0), (0, 1), (1, 0)]  # (lhs comp, rhs comp) over [hi, lo]
        ncomp = 2
    elif scores_mode == "f32r":
        s_dt = F32R
        passes = [(0, 0)]
        ncomp = 1
    else:
        raise ValueError(scores_mode)

    def split(hi, lo, src):
        """hi = ACT cast; lo = DVE (src - hi) rounded."""
        nc.scalar.copy(hi, src)
        nc.vector.tensor_sub(lo, src, hi)

    with tile.TileContext(nc) as tc:
      def emit_body():
            # ---------------- constants ----------------
            const_pool = tc.alloc_tile_pool(name="const", bufs=1)
            ident16 = const_pool.tile([P, P], F16)
            make_identity(nc, ident16[:])
            ident32 = const_pool.tile([P, P], F32)
            make_identity(nc, ident32[:])
            if ncomp == 1:
                ident_r = const_pool.tile([P, P], s_dt)
                nc.vector.tensor_copy(ident_r[:], ident32[:])
                id_s = ident_r
            else:
                id_s = ident16

            def tr8(ps_pool, dst3d, src2d, qi, ident, dt, tag="tp", copy_eng="scalar"):
                """Transpose NBD 128x128 blocks of src2d [P, D] into column qi of
                dst3d [P, NBD, cols] via PSUM + strided copies. 4-byte dtypes
                split into two half-tiles so each stays within one PSUM bank."""
                gsz = NBD if dt in (F16, mybir.dt.bfloat16) else NBD // 2
                for g0 in range(0, NBD, gsz):
                    t = ps_pool.tile([P, gsz * P], dt, tag=tag,
                                     name=f"t_{tag}_{qi}_{g0}")
                    for j in range(gsz):
                        nc.tensor.transpose(t[:, j * P:(j + 1) * P],
                                            src2d[:, (g0 + j) * P:(g0 + j + 1) * P],
                                            ident[:])
                    dst = dst3d[:, g0:g0 + gsz, qi * P:(qi + 1) * P]
                    src = t[:].rearrange("p (j q) -> p j q", j=gsz)
                    if copy_eng == "scalar":
                        nc.scalar.copy(dst, src)
                    else:
                        nc.vector.tensor_copy(dst, src)

            # ---------------- resident: q_projT ----------------
            # qpT[c]: [P, NBD*S]; block db at columns [db*S, (db+1)*S)
            qpT_pool = tc.alloc_tile_pool(name="qpT", bufs=1)
            qpT = [qpT_pool.tile([P, NBD * S], s_dt, name=f"qpT{c}")
                   for c in range(ncomp)]

            # ============ phase 0a: query transpose + W_k + q_projT, interleaved ====
            with tc.tile_pool(name="p0a_sb", bufs=3 if ncomp == 2 else 6) as p0a_sb, \
                 tc.tile_pool(name="p0a_wk", bufs=1) as p0a_wk, \
                 tc.tile_pool(name="p0a_qt", bufs=1) as p0a_qt, \
                 tc.tile_pool(name="p0a_ps", bufs=2, space="PSUM") as p0a_ps, \
                 tc.tile_pool(name="p0a_ps2", bufs=4, space="PSUM") as p0a_ps2:

                qt_c = [p0a_qt.tile([P, NBD * S], s_dt, name=f"qt{c}")
                        for c in range(ncomp)]
                qt3 = [t[:].rearrange("p (j s) -> p j s", j=NBD) for t in qt_c]
                wk_c = [[p0a_wk.tile([P, D], s_dt, name=f"wk{c}_{i}")
                         for i in range(NBD)]
                        for c in range(ncomp)]

                def do_query_tile(qi):
                    q_f32 = p0a_sb.tile([P, D], F32, tag="ld32", name=f"qld{qi}")
                    nc.sync.dma_start(q_f32[:], d_query[qi * P:(qi + 1) * P, :])
                    if ncomp == 2:
                        qh = p0a_sb.tile([P, D], F16, tag="q_hi", name=f"qh{qi}")
                        ql = p0a_sb.tile([P, D], F16, tag="q_lo", name=f"ql{qi}")
                        split(qh[:], ql[:], q_f32[:])
                        for c, src in enumerate([qh[:], ql[:]]):
                            tr8(p0a_ps, qt3[c], src, qi, id_s, s_dt,
                                copy_eng="vector" if c else "scalar")
                    else:
                        # transpose in fp32 (PE has slack here); the f32r
                        # rounding happens on the PSUM->SBUF convert copy
                        tr8(p0a_ps, qt3[0], q_f32[:], qi, ident32, F32,
                            copy_eng="vector")

                def do_qp_chunk(qc):
                    # q_projT[d, qc-cols] = sum_{d'} W_k[d', d] * QT[d', qc-cols]
                    for db in range(NBD):
                        ps = p0a_ps2.tile([P, 512], F32, tag="qp",
                                          name=f"qp{db}_{qc}")
                        n_acc = len(passes) * NBD
                        idx = 0
                        for (lc, rc) in passes:
                            for dpb in range(NBD):
                                nc.tensor.matmul(
                                    ps[:],
                                    wk_c[lc][dpb][:, db * P:(db + 1) * P],
                                    qt_c[rc][:, dpb * S + qc * 512:
                                              dpb * S + (qc + 1) * 512],
                                    start=(idx == 0), stop=(idx == n_acc - 1))
                                idx += 1
                        off = db * S + qc * 512
                        if ncomp == 2:
                            split(qpT[0][:, off:off + 512],
                                  qpT[1][:, off:off + 512], ps[:])
                        else:
                            nc.vector.tensor_copy(qpT[0][:, off:off + 512], ps[:])

                # W_k first (chunk 0 needs all of it), on the gpsimd queue so
                # it streams in parallel with the query loads on sync's queue.
                for i in range(NBD):
                    wk_f32 = p0a_sb.tile([P, D], F32, tag="ldwk",
                                         name=f"wkld{i}", bufs=3)
                    nc.gpsimd.dma_start(wk_f32[:], d_wk[i * P:(i + 1) * P, :])
                    if ncomp == 2:
                        split(wk_c[0][i][:], wk_c[1][i][:], wk_f32[:])
                    else:
                        nc.vector.tensor_copy(wk_c[0][i][:], wk_f32[:])
                for qi in range(4):
                    do_query_tile(qi)
                for qc in range(4):
                    do_qp_chunk(qc)
                    if qc < 3:
                        for qi in range(4 * (qc + 1), 4 * (qc + 2)):
                            do_query_tile(qi)

            # scores PSUM lives from phase 0b (first q-block overlap) onward
            sc_ps = tc.alloc_tile_pool(name="sc_ps", bufs=1, space="PSUM")

            # ---------------- resident: keyT, value, W_oT ----------------
            kT_pool = tc.alloc_tile_pool(name="kT", bufs=1)
            v_pool = tc.alloc_tile_pool(name="v", bufs=1)
            wo_pool = tc.alloc_tile_pool(name="wo", bufs=1)
            kT = [kT_pool.tile([P, NBD * S], s_dt, name=f"kT{c}")
                  for c in range(ncomp)]
            kT3 = [t[:].rearrange("p (j s) -> p j s", j=NBD) for t in kT]
            vv = [v_pool.tile([P, D], F16, name=f"v_{i}") for i in range(NBS)]
            woT = wo_pool.tile([P, NBD * D], F16, name="woT")
            woT3 = woT[:].rearrange("p (j o) -> p j o", j=NBD)

            # softmax-state pools (used from phase 0b for q-block 0)
            exp_sb = tc.alloc_tile_pool(name="exp_sb", bufs=2)
            st_sb = tc.alloc_tile_pool(name="st_sb", bufs=2)

            state = {}

            def head_mm_chunk(qb, kc, scores, batch=None):
                """Emit the scores matmuls for 512-col chunk kc of q-block qb.

                batch=None emits all len(passes)*NBD accumulating matmuls;
                batch=i emits only pass i's NBD matmuls (same accumulation
                group, split for interleaving as PE filler inside tail())."""
                q0 = qb * P
                n_acc = len(passes) * NBD
                for pi, (lc, rc) in enumerate(passes):
                    if batch is not None and pi != batch:
                        continue
                    for db in range(NBD):
                        idx = pi * NBD + db
                        nc.tensor.matmul(
                            scores[:, kc * 512:(kc + 1) * 512],
                            qpT[lc][:, db * S + q0:db * S + q0 + P],
                            kT[rc][:, db * S + kc * 512:db * S + (kc + 1) * 512],
                            start=(idx == 0), stop=(idx == n_acc - 1))

            def head_mm(qb, chunks=None):
                if qb not in state:
                    state[qb] = {"scores": sc_ps.tile([P, S], F32, tag="scores",
                                                      name=f"scores{qb}")}
                scores = state[qb]["scores"]
                for kc in (range(S // 512) if chunks is None else chunks):
                    head_mm_chunk(qb, kc, scores)

            def head_softmax(qb, max_eng="vector"):
                """Row max (negated) + exp-with-rowsum. The reciprocal is
                emitted separately (head_recip) so it can sit AFTER the
                previous block's DVE evacuations in the in-order DVE queue."""
                st = state[qb]
                scores = st["scores"]
                neg_max = st_sb.tile([P, 1], F32, tag="negmax", name=f"negmax{qb}")
                eng = nc.gpsimd if max_eng == "pool" else nc.vector
                eng.reduce_max(neg_max[:], scores[:],
                               axis=mybir.AxisListType.X, negate=True)
                rowsum = st_sb.tile([P, 1], F32, tag="rowsum", name=f"rowsum{qb}")
                expv = exp_sb.tile([P, S], F16, tag="expv", name=f"expv{qb}")
                nc.scalar.activation(expv[:], scores[:],
                                     mybir.ActivationFunctionType.Exp,
                                     bias=neg_max[:], scale=1.0,
                                     accum_out=rowsum[:])
                st["expv"] = expv
                st["rowsum"] = rowsum

            def head_recip(qb):
                st = state[qb]
                recip = st_sb.tile([P, 1], F32, tag="recip", name=f"recip{qb}")
                nc.vector.reciprocal(recip[:], st["rowsum"][:])
                st["recip"] = recip

            # ============ phase 0b: keyT build overlapped with scores(0) ============
            with tc.tile_pool(name="p0b_sb", bufs=3 if ncomp == 1 else 2) as p0b_sb, \
                 tc.tile_pool(name="p0b_ps", bufs=2, space="PSUM") as p0b_ps:

                scores0 = sc_ps.tile([P, S], F32, tag="scores", name="scores_0")
                state[0] = {"scores": scores0}

                def do_wo_tile(oi):
                    wo_f16 = p0b_sb.tile([P, D], F16, tag="ld16", name=f"wold{oi}")
                    nc.gpsimd.dma_start(wo_f16[:], d_wo[oi * P:(oi + 1) * P, :])
                    t16 = p0b_ps.tile([P, NBD * P], F16, tag="tp16",
                                      name=f"twoT{oi}", bufs=1)
                    for j in range(NBD):
                        nc.tensor.transpose(t16[:, j * P:(j + 1) * P],
                                            wo_f16[:, j * P:(j + 1) * P],
                                            ident16[:])
                    nc.scalar.copy(woT3[:, :, oi * P:(oi + 1) * P],
                                   t16[:].rearrange("p (j q) -> p j q", j=NBD))

                for kc in range(4):
                    for ki in range(4 * kc, 4 * (kc + 1)):
                        k_f32 = p0b_sb.tile([P, D], F32, tag="ld32",
                                            name=f"kld{ki}")
                        # alternate queues: the key stream is 0b's critical
                        # path, two queues halve its serial latency
                        keng = nc.sync if ki % 2 == 0 else nc.gpsimd
                        keng.dma_start(k_f32[:], d_key[ki * P:(ki + 1) * P, :])
                        if ncomp == 2:
                            kh = p0b_sb.tile([P, D], F16, tag="k_hi",
                                             name=f"kh{ki}")
                            kl = p0b_sb.tile([P, D], F16, tag="k_lo",
                                             name=f"kl{ki}")
                            split(kh[:], kl[:], k_f32[:])
                            for c, src in enumerate([kh[:], kl[:]]):
                                tr8(p0b_ps, kT3[c], src, ki, id_s, s_dt,
                                    copy_eng="vector" if c else "scalar")
                        else:
                            tr8(p0b_ps, kT3[0], k_f32[:], ki, ident32, F32,
                                copy_eng="vector")
                    head_mm_chunk(0, kc, scores0)

                head_softmax(0)
                head_recip(0)

                # value lands directly in its resident fp16 tiles (gpsimd
                # queue, behind the odd key tiles); needed only by tail(0),
                # ~40us in. W_o follows.
                for ki in range(NBS):
                    nc.gpsimd.dma_start(vv[ki][:],
                                        d_value[ki * P:(ki + 1) * P, :])
                for oi in range(NBD):
                    do_wo_tile(oi)

            # ============ main loop over q blocks (software-pipelined) ============
            # ptp (2KB f16) and xp (2KB f32) share one 2-buffer PSUM ring:
            # their lifetimes interleave (ptp g0/g1 -> xp dh0/dh1), so two
            # banks double-buffer both, and PSUM stays at 8 banks total.
            trx_ps = tc.alloc_tile_pool(name="trx_ps", bufs=2, space="PSUM")
            tr_ps = trx_ps
            x_ps = trx_ps
            o_ps = tc.alloc_tile_pool(name="o_ps", bufs=2, space="PSUM")
            pt_sb = tc.alloc_tile_pool(name="pt_sb", bufs=4)
            xt_sb = tc.alloc_tile_pool(name="xt_sb", bufs=2)
            out_sb = tc.alloc_tile_pool(name="out_sb", bufs=2)

            def tail_pt(qb):
                """Transpose exp(scores) for q-block qb into pT (k on
                partitions): 8 blocks per PSUM bank, evacuated on ACT."""
                st = state[qb]
                expv = st["expv"]
                pts = []
                for g in range(2):
                    ptp = tr_ps.tile([P, 8 * P], F16, tag="trx",
                                     name=f"ptp{qb}_{g}")
                    for j in range(8):
                        kb = g * 8 + j
                        nc.tensor.transpose(
                            ptp[:, j * P:(j + 1) * P],
                            expv[:, kb * P:(kb + 1) * P], ident16[:])
                    pt = pt_sb.tile([P, 8 * P], F16, tag="pt", name=f"pt{qb}_{g}")
                    nc.scalar.copy(pt[:], ptp[:])
                    pts.append(pt)
                st["pts"] = pts

            def tail_rest_xt(qb, filler=None):
                """xT = V.T-contracted with pT (no transpose needed: v is the
                stationary operand in natural [k, d] layout), then
                out[q, o] = xT.T @ woT with the 1/rowsum folded into the
                final PSUM evacuation (q is the partition dim there)."""
                st = state.pop(qb)
                pts, recip = st["pts"], st["recip"]

                # xT[d, q] in two halves of d; 4 d-blocks per PSUM bank
                xts = []
                for h in range(2):
                    xp = x_ps.tile([P, 4 * P], F32, tag="trx",
                                   name=f"xp{qb}_{h}")
                    for dj in range(4):
                        db = h * 4 + dj
                        for kb in range(NBS):
                            nc.tensor.matmul(
                                xp[:, dj * P:(dj + 1) * P],
                                vv[kb][:, db * P:(db + 1) * P],
                                pts[kb // 8][:, (kb % 8) * P:(kb % 8 + 1) * P],
                                start=(kb == 0), stop=(kb == NBS - 1))
                    xt = xt_sb.tile([P, 4 * P], F16, tag="xt",
                                    name=f"xt{qb}_{h}")
                    nc.scalar.copy(xt[:], xp[:])
                    xts.append(xt)
                    if h == 0 and filler is not None:
                        filler()

                # out = xT.T @ woT, o in halves; scale by recip on evacuation
                ops = [o_ps.tile([P, 512], F32, tag="op", name=f"op{qb}_{i}")
                       for i in range(2)]
                for db in range(NBD):
                    lhs = xts[db // 4][:, (db % 4) * P:(db % 4 + 1) * P]
                    for oh in range(2):
                        nc.tensor.matmul(
                            ops[oh][:], lhs,
                            woT[:, db * D + oh * 512:db * D + (oh + 1) * 512],
                            start=(db == 0), stop=(db == NBD - 1))
                q0 = qb * P
                for oh in range(2):
                    osb = out_sb.tile([P, 512], F32, tag="osb",
                                      name=f"osb{qb}_{oh}")
                    nc.vector.tensor_scalar_mul(osb[:], ops[oh][:], recip[:])
                    nc.sync.dma_start(
                        d_out[q0:q0 + P, oh * 512:(oh + 1) * 512], osb[:])

            def tail_rest_tr(qb, filler=None):
                """Classic tail: x = pT.T @ v (512-col moving dim), normalize
                on evacuation, PE-transpose x, then out = xT.T @ woT.
                `filler` emits PE work (next block's scores chunks) after the
                first x half to cover the single-buffer PSUM evacuation."""
                st = state.pop(qb)
                pts, recip = st["pts"], st["recip"]
                xn = xt_sb.tile([P, D], F16, tag="xn", name=f"xn{qb}")
                for dh in range(2):
                    xp = x_ps.tile([P, 512], F32, tag="trx", name=f"xp{qb}_{dh}")
                    for kb in range(NBS):
                        nc.tensor.matmul(
                            xp[:],
                            pts[kb // 8][:, (kb % 8) * P:(kb % 8 + 1) * P],
                            vv[kb][:, dh * 512:(dh + 1) * 512],
                            start=(kb == 0), stop=(kb == NBS - 1))
                    nc.vector.tensor_scalar_mul(
                        xn[:, dh * 512:(dh + 1) * 512], xp[:], recip[:])
                    if dh == 0 and filler is not None:
                        filler()
                xtp = tr_ps.tile([P, 8 * P], F16, tag="trx", name=f"xtp{qb}")
                for j in range(NBD):
                    nc.tensor.transpose(xtp[:, j * P:(j + 1) * P],
                                        xn[:, j * P:(j + 1) * P], ident16[:])
                xt = xt_sb.tile([P, 8 * P], F16, tag="xt", name=f"xt{qb}")
                nc.scalar.copy(xt[:], xtp[:])
                ops = [o_ps.tile([P, 512], F32, tag="op", name=f"op{qb}_{i}")
                       for i in range(2)]
                for db in range(NBD):
                    lhs = xt[:, db * P:(db + 1) * P]
                    for oh in range(2):
                        nc.tensor.matmul(
                            ops[oh][:], lhs,
                            woT[:, db * D + oh * 512:db * D + (oh + 1) * 512],
                            start=(db == 0), stop=(db == NBD - 1))
                q0 = qb * P
                for oh in range(2):
                    osb = out_sb.tile([P, 512], F32, tag="osb",
                                      name=f"osb{qb}_{oh}")
                    nc.vector.tensor_copy(osb[:], ops[oh][:])
                    nc.sync.dma_start(
                        d_out[q0:q0 + P, oh * 512:(oh + 1) * 512], osb[:])

            tail_rest = tail_rest_xt if tail_mode == "xt" else tail_rest_tr

            for qb in range(1, NBS + 1):
                if qb < NBS:
                    head_mm(qb)
                    tail_pt(qb - 1)
                    # DVE max first (its scores input is ready before the
                    # tail's evacuations need DVE), reciprocal emitted after
                    # the tail so it can't head-of-line-block the DVE queue.
                    head_softmax(qb)
                    tail_rest(qb - 1)
                    head_recip(qb)
                else:
                    tail_pt(qb - 1)
                    tail_rest(qb - 1)

            out_sb.release()
            xt_sb.release()
            pt_sb.release()
            o_ps.release()
            trx_ps.release()
            st_sb.release()
            exp_sb.release()
            wo_pool.release()
            v_pool.release()
            kT_pool.release()
            sc_ps.release()
            qpT_pool.release()
            const_pool.release()


      for _rep in range(repeat):
          emit_body()

    nc.compile()
    return nc


_NC_CACHE = {}


def _get_nc():
    if "nc" not in _NC_CACHE:
        _NC_CACHE["nc"] = build_nc()
    return _NC_CACHE["nc"]


def make_in_maps(query, key, value, W_k, W_o):
    value16 = value.astype(np.float16)
    W_o16 = W_o.astype(np.float16)
    return [
        {"query": query[b], "key": key[b], "value16": value16[b],
         "W_k": W_k, "W_o16": W_o16}
        for b in range(NC)
    ]


def _numpy_fallback(query, key, value, cell_mask, seq_mask, W_k, b_k, W_o, b_o):
    out = np.empty((query.shape[0], S, D), dtype=np.float32)
    for b in range(query.shape[0]):
        kp = key[b].astype(np.float64) @ W_k.astype(np.float64).T + b_k
        s = query[b].astype(np.float64) @ kp.T
        s = s + np.log(cell_mask[b]) + np.log(seq_mask[b])[None, :]
        s -= s.max(1, keepdims=True)
        e = np.exp(s)
        p = e / e.sum(1, keepdims=True)
        x = p @ value[b].astype(np.float64)
        out[b] = (x @ W_o.astype(np.float64).T + b_o).astype(np.float32)
    return out


def kernel(query, key, value, cell_mask, seq_mask, W_k, b_k, W_o, b_o):
    query = np.ascontiguousarray(query, dtype=np.float32)
    key = np.ascontiguousarray(key, dtype=np.float32)
    value = np.ascontiguousarray(value, dtype=np.float32)
    W_k = np.ascontiguousarray(W_k, dtype=np.float32)
    W_o = np.ascontiguousarray(W_o, dtype=np.float32)

    # masks are all-ones per the problem spec -> log-mask bias is exactly 0.
    # b_k shifts every score row by a constant -> softmax-invariant (exact).
    if not (np.all(np.asarray(cell_mask) == 1.0)
            and np.all(np.asarray(seq_mask) == 1.0)):
        return _numpy_fallback(np.asarray(query), np.asarray(key),
                               np.asarray(value), np.asarray(cell_mask),
                               np.asarray(seq_mask), W_k,
                               np.asarray(b_k), W_o, np.asarray(b_o))

    nc = _get_nc()
    in_maps = make_in_maps(query, key, value, W_k, W_o)
    res = run_bass_kernel_spmd(nc, in_maps, core_ids=list(range(NC)))
    out = np.stack([res.results[b]["out"] for b in range(NC)])
    if b_o is not None and np.any(np.asarray(b_o) != 0.0):
        out = out + np.asarray(b_o, dtype=np.float32)[None, None, :]
    return out

